# revision 19
# baseline (speedup 1.0000x reference)
"""Trainium2 Bass kernel for nn_ClassBasedDecoder (moe_routing).

Reference computation:
    p_class = input @ cls_w.T + cls_b                     [NTOK, NCLS]
    d       = input[within_batch_idx]                     [NCLS, TPC, NHID]
    emb     = word_emb[cluster]                           [NCLS, CSIZE, NHID]
    p_words = einsum('cth,csh->cts', d, emb) + word_bias[cluster][:,None,:]

Sharding: expert/class parallel — 8 classes per NeuronCore. The MoE
"all-to-all" dispatch is performed on the host (numpy gather); since
within_batch_idx is a permutation of all rows, the per-core gathered rows
cover all tokens exactly once, so p_class is computed on the gathered rows
too and scattered back on the host.

Device layout: contraction (hid) on partitions. Host pre-transposes the
gathered activations and embeddings into K-tiled layouts so every DMA is a
single contiguous block. The grouped GEMM accumulates 8 K-tiles into PSUM
per (class, 512-word half); per-token/word biases are added on the host
(0.5% of the FLOPs). Outputs leave the device as fp16 (values are O(10);
fp16 rounding adds ~2e-4 rel err vs the 2e-3 bf16 matmul error) to halve
store traffic. p_class runs first so the PE has work while the first
embedding pair streams in, and the kernel tail is only the last class's
matmuls + store.

DMA structure is constrained by this walrus build: an instruction may
carry at most ONE sync wait. Tile emits a second wait on an instruction
whenever its DMA-semaphore lane (8 HWDGE + 8 SWDGE) or destination SBUF
slot is reused, so: all embedding slots are resident (no slot reuse),
loads use <=8 distinct HWDGE lanes, stores use SWDGE lanes, every
PSUM->SBUF copy goes to the scalar engine (stores then wait on a single
semaphore), and an `ldweights` probe after each embedding load makes the
PE observe the DMA semaphore so the real matmuls only carry their
PSUM-slot wait. Tile's final multi-wait Drain is split into standalone
EventSemaphore waits in a post-pass.
"""

import os
import sys

import numpy as np

for _p in (
    "/root/.axon_site",
    "/root/.axon_site/_ro/trn_rl_repo",
    "/root/.axon_site/_ro/pypackages",
    "/opt/trn_rl_repo",
    "/opt/pypackages",
):
    if os.path.isdir(_p) and _p not in sys.path:
        sys.path.append(_p)

NHID = 1024
NWORDS = 65536
NCLS = 64
CSIZE = NWORDS // NCLS  # 1024
NTOK = 8192
TPC = NTOK // NCLS      # 128
NCORES = 8
CPC = NCLS // NCORES    # 8 classes per core
TOK = CPC * TPC         # 1024 tokens per core
KT = NHID // 128        # 8 k-tiles
NF = 512                # matmul moving free dim / PSUM bank
NHALF = CSIZE // NF     # 2
NPAIR = CPC // 2        # 4 class pairs (one emb load / pw store per pair)

# consts_sb column offsets (elements)
OFF_DT = 0                      # dT: KT*TOK = 8192
OFF_CW = OFF_DT + KT * TOK      # cwT: KT*NCLS = 512
NCONST = OFF_CW + KT * NCLS     # 8704

MODE = os.environ.get("KMODE", "bf16")

_CACHE = {}


def _build(mode):
    import concourse.bass as bass
    import concourse.mybir as mybir
    import concourse.tile as tile
    from bass_rust import add_dep_helper

    f32 = mybir.dt.float32
    f16 = mybir.dt.float16
    in_dt = mybir.dt.bfloat16 if mode == "bf16" else f32
    emb_bufs = 1  # each chunk has its own tag; all resident

    def mm(ap):
        return ap.bitcast(mybir.dt.float32r) if mode == "f32r" else ap

    nc = bass.Bass("TRN2", target_bir_lowering=False, debug=False)

    consts = nc.dram_tensor("consts", [128, NCONST], in_dt, kind="ExternalInput").ap()
    # flat buffer of per-chunk blocks, each [128, nc, KT, CSIZE] (p-major)
    embT = nc.dram_tensor("embT", [CPC * 128 * KT * CSIZE], in_dt, kind="ExternalInput").ap()
    pw = nc.dram_tensor("pw", [CPC, TPC, NHALF, NF], f16, kind="ExternalOutput").ap()
    pcT = nc.dram_tensor("pcT", [NCLS, NHALF, NF], f16, kind="ExternalOutput").ap()

    with tile.TileContext(nc) as tc:
        with (
            tc.tile_pool(name="const", bufs=1) as const_pool,
            tc.tile_pool(name="emb", bufs=emb_bufs) as emb_pool,
            tc.tile_pool(name="psum", bufs=4, space="PSUM") as psum_pool,
        ):
            consts_sb = const_pool.tile([128, NCONST], in_dt)
            nc.sync.dma_start(consts_sb[:], consts[:])

            out_sb = const_pool.tile([128, CPC, NHALF, NF], f16)
            oc_sb = const_pool.tile([NCLS, NHALF, NF], f16)

            def dT_lhsT(k, c):
                s = OFF_DT + k * TOK + c * TPC
                return consts_sb[:, s:s + TPC]

            def dT_rhs(k, n):
                s = OFF_DT + k * TOK + n * NF
                return consts_sb[:, s:s + NF]

            def cw_lhsT(k):
                s = OFF_CW + k * NCLS
                return consts_sb[:, s:s + NCLS]

            # ---- p_class first: only needs consts; runs while emb streams in
            # p_classT[cls, tok] = cls_w @ d.T
            for n in range(TOK // NF):
                ps = psum_pool.tile([128, NF], f32)
                for k in range(KT):
                    nc.tensor.matmul(
                        ps[:NCLS],
                        mm(cw_lhsT(k)),
                        mm(dT_rhs(k, n)),
                        start=(k == 0),
                        stop=(k == KT - 1),
                    )
                nc.scalar.copy(oc_sb[:, n, :], ps[:NCLS])

            # ---- grouped GEMM, one emb load + one pw store per chunk.
            # Small chunks at the end shrink the post-last-byte tail.
            CHUNKS = [2, 1, 1, 1, 1, 1, 1]
            assert sum(CHUNKS) == CPC
            c0 = 0
            for ci, ncls_chunk in enumerate(CHUNKS):
                emb_sb = emb_pool.tile([128, ncls_chunk, KT, CSIZE], in_dt,
                                       tag=f"emb{c0}")
                blk = 128 * KT * CSIZE
                src = embT[c0 * blk:(c0 + ncls_chunk) * blk].rearrange(
                    "(p c k s) -> p c k s", p=128, c=ncls_chunk, k=KT, s=CSIZE
                )
                nc.sync.dma_start(emb_sb[:], src)
                # PE-side probe: observes the emb DMA semaphore so the real
                # matmuls below need only their single PSUM-slot wait.
                probe = nc.tensor.ldweights(emb_sb[:, 0, 0, 0:TPC])
                for cc in range(ncls_chunk):
                    c = c0 + cc
                    for n in range(NHALF):
                        ps = psum_pool.tile([128, NF], f32)
                        for k in range(KT):
                            mm_inst = nc.tensor.matmul(
                                ps[:],
                                mm(dT_lhsT(k, c)),
                                mm(emb_sb[:, cc, k, n * NF:(n + 1) * NF]),
                                start=(k == 0),
                                stop=(k == KT - 1),
                            )
                            if cc == 0 and n == 0 and k == 0:
                                add_dep_helper(
                                    mm_inst.ins, probe.ins, sync=False,
                                    reason="first matmul of chunk after emb probe",
                                )
                        nc.scalar.copy(out_sb[:, c, n, :], ps[:])
                nc.gpsimd.dma_start(
                    pw[c0:c0 + ncls_chunk].rearrange("c t n s -> t c n s"),
                    out_sb[:, c0:c0 + ncls_chunk],
                )
                c0 += ncls_chunk
            nc.gpsimd.dma_start(pcT[:], oc_sb[:])

    _split_multiwait_drains(nc, mybir)
    return nc


def _split_multiwait_drains(nc, mybir):
    """This walrus build rejects instructions with >1 sync wait. Tile's final
    Drain carries one wait per semaphore; split them into standalone
    EventSemaphore waits appended to the preceding block (which executes
    immediately before the end-block drain)."""
    f = nc.m.functions[0]
    blocks = list(f.blocks)
    by_name = {b.name: b for b in blocks}
    for b in blocks:
        insts = list(b.instructions)
        if not insts:
            continue
        first = insts[0]
        if type(first).__name__ != "InstDrain":
            continue
        si = first.sync_info
        if si is None or len(si.on_wait) <= 1:
            continue
        assert b.name.endswith("_end"), b.name
        body = by_name[b.name[:-len("_end")]]
        for j, w in enumerate(si.on_wait):
            ev = mybir.InstEventSemaphore(
                name=f"{first.name}-wait{j}", engine=first.engine
            )
            ev.sync_info = mybir.SyncInfo(on_wait=[w], on_update=[])
            body.add_instruction(ev)
        first.sync_info = mybir.SyncInfo(on_wait=[], on_update=list(si.on_update))


def _get_nc(mode):
    if mode not in _CACHE:
        _CACHE[mode] = _build(mode)
    return _CACHE[mode]


def _prep_inputs(input, cls_w, word_emb, within_batch_idx, cluster, mode):
    """Host-side shard + layout. Returns (in_maps, row_ids_per_core)."""
    if mode == "bf16":
        import ml_dtypes
        np_in = ml_dtypes.bfloat16
    else:
        np_in = np.float32

    input = np.asarray(input, np.float32)
    cls_w = np.asarray(cls_w, np.float32)
    word_emb = np.asarray(word_emb, np.float32)
    idx = np.asarray(within_batch_idx).astype(np.int64)
    clu = np.asarray(cluster).astype(np.int64)

    # replicated cls decoder, [hid, cls] k-tiled: [128, KT, NCLS]
    cwT_h = np.ascontiguousarray(cls_w.T.reshape(KT, 128, NCLS).transpose(1, 0, 2))

    in_maps = []
    rows_per_core = []
    for i in range(NCORES):
        crows = idx[i * CPC:(i + 1) * CPC].reshape(-1)          # [TOK]
        rows_per_core.append(crows)
        d = input[crows]                                        # [TOK, NHID]
        dT_h = d.T.reshape(KT, 128, TOK).transpose(1, 0, 2)     # [128, KT, TOK]

        wrows = clu[i * CPC:(i + 1) * CPC].reshape(-1)          # [CPC*CSIZE]
        e = word_emb[wrows]                                     # [CPC*CSIZE, NHID]
        # flat per-chunk blocks, each [128, nc, KT, CSIZE] (p-major)
        chunks = []
        c0 = 0
        for ncls_chunk in (2, 1, 1, 1, 1, 1, 1):
            ec = e[c0 * CSIZE:(c0 + ncls_chunk) * CSIZE]        # [nc*CSIZE, NHID]
            chunks.append(np.ascontiguousarray(
                ec.reshape(ncls_chunk, CSIZE, KT, 128).transpose(3, 0, 2, 1)
            ).reshape(-1))
            c0 += ncls_chunk
        embT_h = np.concatenate(chunks).astype(np_in)

        consts_h = np.empty((128, NCONST), np.float32)
        consts_h[:, OFF_DT:OFF_DT + KT * TOK] = dT_h.reshape(128, KT * TOK)
        consts_h[:, OFF_CW:OFF_CW + KT * NCLS] = cwT_h.reshape(128, KT * NCLS)

        in_maps.append({
            "consts": consts_h.astype(np_in),
            "embT": embT_h,
        })
    return in_maps, rows_per_core


def _run(in_maps, mode, trace=False, trace_cores=None):
    from concourse.bass_utils import run_bass_kernel_spmd

    nc = _get_nc(mode)
    return run_bass_kernel_spmd(
        nc, in_maps, list(range(NCORES)), trace=trace, trace_cores=trace_cores
    )


def kernel(input, cls_w, cls_b, word_emb, word_bias,
           within_batch_idx, cluster, _trace=False, _mode=None, _trace_cores=None):
    mode = _mode or MODE
    in_maps, rows_per_core = _prep_inputs(
        input, cls_w, word_emb, within_batch_idx, cluster, mode)

    res = _run(in_maps, mode, trace=_trace, trace_cores=_trace_cores)

    p_words = np.empty((NCLS, TPC, CSIZE), np.float32)
    p_class = np.empty((NTOK, NCLS), np.float32)
    for i in range(NCORES):
        out = res.results[i]
        # pw [CPC, TPC, NHALF, NF] -> [CPC, TPC, CSIZE]
        p_words[i * CPC:(i + 1) * CPC] = (
            out["pw"].astype(np.float32).reshape(CPC, TPC, CSIZE)
        )
        # pcT [NCLS, NHALF, NF] -> [NCLS, TOK] -> scatter rows
        pct = out["pcT"].astype(np.float32).reshape(NCLS, TOK)
        p_class[rows_per_core[i]] = pct.T

    # biases (0.5% of FLOPs) applied on host in fp32
    word_bias = np.asarray(word_bias, np.float32)
    cls_b = np.asarray(cls_b, np.float32)
    clu = np.asarray(cluster).astype(np.int64)
    p_words += word_bias[clu][:, None, :]
    p_class += cls_b[None, :]

    # safety net: if within_batch_idx is not a permutation of all rows,
    # p_class rows outside it were never computed on-device — fill densely.
    idx_flat = np.asarray(within_batch_idx).astype(np.int64).reshape(-1)
    if np.unique(idx_flat).size != NTOK:
        p_class = (np.asarray(input, np.float32) @
                   np.asarray(cls_w, np.float32).T + cls_b[None, :])

    kernel.last_results = res
    return p_class, p_words


# revision 20
# speedup vs baseline: 1.0020x; 1.0020x over previous
"""Trainium2 Bass kernel for nn_ClassBasedDecoder (moe_routing).

Reference computation:
    p_class = input @ cls_w.T + cls_b                     [NTOK, NCLS]
    d       = input[within_batch_idx]                     [NCLS, TPC, NHID]
    emb     = word_emb[cluster]                           [NCLS, CSIZE, NHID]
    p_words = einsum('cth,csh->cts', d, emb) + word_bias[cluster][:,None,:]

Sharding: expert/class parallel — 8 classes per NeuronCore. The MoE
"all-to-all" dispatch is performed on the host (numpy gather); since
within_batch_idx is a permutation of all rows, the per-core gathered rows
cover all tokens exactly once, so p_class is computed on the gathered rows
too and scattered back on the host.

Device layout: contraction (hid) on partitions. Host pre-transposes the
gathered activations and embeddings into K-tiled layouts so every DMA is a
single contiguous block. The grouped GEMM accumulates 8 K-tiles into PSUM
per (class, 512-word half); per-token/word biases are added on the host
(0.5% of the FLOPs). Outputs leave the device as fp16 (values are O(10);
fp16 rounding adds ~2e-4 rel err vs the 2e-3 bf16 matmul error) to halve
store traffic. p_class runs first so the PE has work while the first
embedding pair streams in, and the kernel tail is only the last class's
matmuls + store.

DMA structure is constrained by this walrus build: an instruction may
carry at most ONE sync wait. Tile emits a second wait on an instruction
whenever its DMA-semaphore lane (8 HWDGE + 8 SWDGE) or destination SBUF
slot is reused, so: all embedding slots are resident (no slot reuse),
loads use <=8 distinct HWDGE lanes, stores use SWDGE lanes, every
PSUM->SBUF copy goes to the scalar engine (stores then wait on a single
semaphore), and an `ldweights` probe after each embedding load makes the
PE observe the DMA semaphore so the real matmuls only carry their
PSUM-slot wait. Tile's final multi-wait Drain is split into standalone
EventSemaphore waits in a post-pass.
"""

import os
import sys

import numpy as np

for _p in (
    "/root/.axon_site",
    "/root/.axon_site/_ro/trn_rl_repo",
    "/root/.axon_site/_ro/pypackages",
    "/opt/trn_rl_repo",
    "/opt/pypackages",
):
    if os.path.isdir(_p) and _p not in sys.path:
        sys.path.append(_p)

NHID = 1024
NWORDS = 65536
NCLS = 64
CSIZE = NWORDS // NCLS  # 1024
NTOK = 8192
TPC = NTOK // NCLS      # 128
NCORES = 8
CPC = NCLS // NCORES    # 8 classes per core
TOK = CPC * TPC         # 1024 tokens per core
KT = NHID // 128        # 8 k-tiles
NF = 512                # matmul moving free dim / PSUM bank
NHALF = CSIZE // NF     # 2
NPAIR = CPC // 2        # 4 class pairs (one emb load / pw store per pair)

# consts_sb column offsets (elements)
OFF_DT = 0                      # dT: KT*TOK = 8192
OFF_CW = OFF_DT + KT * TOK      # cwT: KT*NCLS = 512
NCONST = OFF_CW + KT * NCLS     # 8704

MODE = os.environ.get("KMODE", "bf16")

# emb streamed in per-chunk loads (classes per chunk); small tail chunks
CHUNKS = (2, 1, 1, 1, 1, 1, 1)

_CACHE = {}


def _build(mode):
    import concourse.bass as bass
    import concourse.mybir as mybir
    import concourse.tile as tile
    from bass_rust import add_dep_helper

    f32 = mybir.dt.float32
    f16 = mybir.dt.float16
    in_dt = mybir.dt.bfloat16 if mode == "bf16" else f32
    emb_bufs = 1  # each chunk has its own tag; all resident

    def mm(ap):
        return ap.bitcast(mybir.dt.float32r) if mode == "f32r" else ap

    nc = bass.Bass("TRN2", target_bir_lowering=False, debug=False)

    consts = nc.dram_tensor("consts", [128, NCONST], in_dt, kind="ExternalInput").ap()
    # flat buffer of per-chunk blocks, each [128, nc, KT, CSIZE] (p-major)
    embT = nc.dram_tensor("embT", [CPC * 128 * KT * CSIZE], in_dt, kind="ExternalInput").ap()
    pw = nc.dram_tensor("pw", [CPC, TPC, NHALF, NF], f16, kind="ExternalOutput").ap()
    pcT = nc.dram_tensor("pcT", [NCLS, NHALF, NF], f16, kind="ExternalOutput").ap()

    with tile.TileContext(nc) as tc:
        with (
            tc.tile_pool(name="const", bufs=1) as const_pool,
            tc.tile_pool(name="emb", bufs=emb_bufs) as emb_pool,
            tc.tile_pool(name="psum", bufs=4, space="PSUM") as psum_pool,
        ):
            consts_sb = const_pool.tile([128, NCONST], in_dt)
            nc.sync.dma_start(consts_sb[:], consts[:])

            out_sb = const_pool.tile([128, CPC, NHALF, NF], f16)
            oc_sb = const_pool.tile([NCLS, NHALF, NF], f16)

            def dT_lhsT(k, c):
                s = OFF_DT + k * TOK + c * TPC
                return consts_sb[:, s:s + TPC]

            def dT_rhs(k, n):
                s = OFF_DT + k * TOK + n * NF
                return consts_sb[:, s:s + NF]

            def cw_lhsT(k):
                s = OFF_CW + k * NCLS
                return consts_sb[:, s:s + NCLS]

            # ---- p_class first: only needs consts; runs while emb streams in
            # p_classT[cls, tok] = cls_w @ d.T
            for n in range(TOK // NF):
                ps = psum_pool.tile([128, NF], f32)
                for k in range(KT):
                    nc.tensor.matmul(
                        ps[:NCLS],
                        mm(cw_lhsT(k)),
                        mm(dT_rhs(k, n)),
                        start=(k == 0),
                        stop=(k == KT - 1),
                    )
                nc.scalar.copy(oc_sb[:, n, :], ps[:NCLS])

            # ---- grouped GEMM, one emb load + one pw store per chunk.
            # Small chunks at the end shrink the post-last-byte tail.
            assert sum(CHUNKS) == CPC
            c0 = 0
            for ci, ncls_chunk in enumerate(CHUNKS):
                emb_sb = emb_pool.tile([128, ncls_chunk, KT, CSIZE], in_dt,
                                       tag=f"emb{c0}")
                blk = 128 * KT * CSIZE
                src = embT[c0 * blk:(c0 + ncls_chunk) * blk].rearrange(
                    "(p c k s) -> p c k s", p=128, c=ncls_chunk, k=KT, s=CSIZE
                )
                nc.sync.dma_start(emb_sb[:], src)
                # PE-side probe: observes the emb DMA semaphore so the real
                # matmuls below need only their single PSUM-slot wait.
                probe = nc.tensor.ldweights(emb_sb[:, 0, 0, 0:TPC])
                for cc in range(ncls_chunk):
                    c = c0 + cc
                    for n in range(NHALF):
                        ps = psum_pool.tile([128, NF], f32)
                        for k in range(KT):
                            mm_inst = nc.tensor.matmul(
                                ps[:],
                                mm(dT_lhsT(k, c)),
                                mm(emb_sb[:, cc, k, n * NF:(n + 1) * NF]),
                                start=(k == 0),
                                stop=(k == KT - 1),
                            )
                            if cc == 0 and n == 0 and k == 0:
                                add_dep_helper(
                                    mm_inst.ins, probe.ins, sync=False,
                                    reason="first matmul of chunk after emb probe",
                                )
                        nc.scalar.copy(out_sb[:, c, n, :], ps[:])
                nc.gpsimd.dma_start(
                    pw[c0:c0 + ncls_chunk].rearrange("c t n s -> t c n s"),
                    out_sb[:, c0:c0 + ncls_chunk],
                )
                c0 += ncls_chunk
            nc.gpsimd.dma_start(pcT[:], oc_sb[:])

    _split_multiwait_drains(nc, mybir)
    return nc


def _split_multiwait_drains(nc, mybir):
    """This walrus build rejects instructions with >1 sync wait. Tile's final
    Drain carries one wait per semaphore; split them into standalone
    EventSemaphore waits appended to the preceding block (which executes
    immediately before the end-block drain)."""
    f = nc.m.functions[0]
    blocks = list(f.blocks)
    by_name = {b.name: b for b in blocks}
    for b in blocks:
        insts = list(b.instructions)
        if not insts:
            continue
        first = insts[0]
        if type(first).__name__ != "InstDrain":
            continue
        si = first.sync_info
        if si is None or len(si.on_wait) <= 1:
            continue
        assert b.name.endswith("_end"), b.name
        body = by_name[b.name[:-len("_end")]]
        for j, w in enumerate(si.on_wait):
            ev = mybir.InstEventSemaphore(
                name=f"{first.name}-wait{j}", engine=first.engine
            )
            ev.sync_info = mybir.SyncInfo(on_wait=[w], on_update=[])
            body.add_instruction(ev)
        first.sync_info = mybir.SyncInfo(on_wait=[], on_update=list(si.on_update))


def _get_nc(mode):
    if mode not in _CACHE:
        _CACHE[mode] = _build(mode)
    return _CACHE[mode]


def _prep_inputs(input, cls_w, word_emb, within_batch_idx, cluster, mode):
    """Host-side shard + layout. Returns (in_maps, row_ids_per_core)."""
    if mode == "bf16":
        import ml_dtypes
        np_in = ml_dtypes.bfloat16
    else:
        np_in = np.float32

    input = np.asarray(input, np.float32)
    cls_w = np.asarray(cls_w, np.float32)
    word_emb = np.asarray(word_emb, np.float32)
    idx = np.asarray(within_batch_idx).astype(np.int64)
    clu = np.asarray(cluster).astype(np.int64)

    # replicated cls decoder, [hid, cls] k-tiled: [128, KT, NCLS]
    cwT_h = np.ascontiguousarray(cls_w.T.reshape(KT, 128, NCLS).transpose(1, 0, 2))

    in_maps = []
    rows_per_core = []
    for i in range(NCORES):
        crows = idx[i * CPC:(i + 1) * CPC].reshape(-1)          # [TOK]
        rows_per_core.append(crows)
        d = input[crows]                                        # [TOK, NHID]
        dT_h = d.T.reshape(KT, 128, TOK).transpose(1, 0, 2)     # [128, KT, TOK]

        wrows = clu[i * CPC:(i + 1) * CPC].reshape(-1)          # [CPC*CSIZE]
        e = word_emb[wrows]                                     # [CPC*CSIZE, NHID]
        # flat per-chunk blocks, each [128, nc, KT, CSIZE] (p-major)
        chunks = []
        c0 = 0
        for ncls_chunk in CHUNKS:
            ec = e[c0 * CSIZE:(c0 + ncls_chunk) * CSIZE]        # [nc*CSIZE, NHID]
            chunks.append(np.ascontiguousarray(
                ec.reshape(ncls_chunk, CSIZE, KT, 128).transpose(3, 0, 2, 1)
            ).reshape(-1))
            c0 += ncls_chunk
        embT_h = np.concatenate(chunks).astype(np_in)

        consts_h = np.empty((128, NCONST), np.float32)
        consts_h[:, OFF_DT:OFF_DT + KT * TOK] = dT_h.reshape(128, KT * TOK)
        consts_h[:, OFF_CW:OFF_CW + KT * NCLS] = cwT_h.reshape(128, KT * NCLS)

        in_maps.append({
            "consts": consts_h.astype(np_in),
            "embT": embT_h,
        })
    return in_maps, rows_per_core


def _run(in_maps, mode, trace=False, trace_cores=None):
    from concourse.bass_utils import run_bass_kernel_spmd

    nc = _get_nc(mode)
    return run_bass_kernel_spmd(
        nc, in_maps, list(range(NCORES)), trace=trace, trace_cores=trace_cores
    )


def kernel(input, cls_w, cls_b, word_emb, word_bias,
           within_batch_idx, cluster, _trace=False, _mode=None, _trace_cores=None):
    mode = _mode or MODE
    in_maps, rows_per_core = _prep_inputs(
        input, cls_w, word_emb, within_batch_idx, cluster, mode)

    res = _run(in_maps, mode, trace=_trace, trace_cores=_trace_cores)

    p_words = np.empty((NCLS, TPC, CSIZE), np.float32)
    p_class = np.empty((NTOK, NCLS), np.float32)
    for i in range(NCORES):
        out = res.results[i]
        # pw [CPC, TPC, NHALF, NF] -> [CPC, TPC, CSIZE]
        p_words[i * CPC:(i + 1) * CPC] = (
            out["pw"].astype(np.float32).reshape(CPC, TPC, CSIZE)
        )
        # pcT [NCLS, NHALF, NF] -> [NCLS, TOK] -> scatter rows
        pct = out["pcT"].astype(np.float32).reshape(NCLS, TOK)
        p_class[rows_per_core[i]] = pct.T

    # biases (0.5% of FLOPs) applied on host in fp32
    word_bias = np.asarray(word_bias, np.float32)
    cls_b = np.asarray(cls_b, np.float32)
    clu = np.asarray(cluster).astype(np.int64)
    p_words += word_bias[clu][:, None, :]
    p_class += cls_b[None, :]

    # safety net: if within_batch_idx is not a permutation of all rows,
    # p_class rows outside it were never computed on-device — fill densely.
    idx_flat = np.asarray(within_batch_idx).astype(np.int64).reshape(-1)
    if np.unique(idx_flat).size != NTOK:
        p_class = (np.asarray(input, np.float32) @
                   np.asarray(cls_w, np.float32).T + cls_b[None, :])

    kernel.last_results = res
    return p_class, p_words


# revision 21
# speedup vs baseline: 1.0339x; 1.0318x over previous
"""Trainium2 Bass kernel for nn_ClassBasedDecoder (moe_routing).

Reference computation:
    p_class = input @ cls_w.T + cls_b                     [NTOK, NCLS]
    d       = input[within_batch_idx]                     [NCLS, TPC, NHID]
    emb     = word_emb[cluster]                           [NCLS, CSIZE, NHID]
    p_words = einsum('cth,csh->cts', d, emb) + word_bias[cluster][:,None,:]

Sharding: expert/class parallel — 8 classes per NeuronCore. The MoE
"all-to-all" dispatch is performed on the host (numpy gather); since
within_batch_idx is a permutation of all rows, the per-core gathered rows
cover all tokens exactly once, so p_class is computed on the gathered rows
too and scattered back on the host.

Device layout: contraction (hid) on partitions. Host pre-transposes the
gathered activations and embeddings into K-tiled layouts so every DMA is a
single contiguous block. The grouped GEMM accumulates 8 K-tiles into PSUM
per (class, 512-word half); per-token/word biases are added on the host
(0.5% of the FLOPs). Outputs leave the device as fp16 (values are O(10);
fp16 rounding adds ~2e-4 rel err vs the 2e-3 bf16 matmul error) to halve
store traffic. p_class runs first so the PE has work while the first
embedding pair streams in, and the kernel tail is only the last class's
matmuls + store.

DMA structure is constrained by this walrus build: an instruction may
carry at most ONE sync wait. Tile emits a second wait on an instruction
whenever its DMA-semaphore lane (8 HWDGE + 8 SWDGE) or destination SBUF
slot is reused, so: all embedding slots are resident (no slot reuse),
loads use <=8 distinct HWDGE lanes, stores use SWDGE lanes, every
PSUM->SBUF copy goes to the scalar engine (stores then wait on a single
semaphore), and an `ldweights` probe after each embedding load makes the
PE observe the DMA semaphore so the real matmuls only carry their
PSUM-slot wait. Tile's final multi-wait Drain is split into standalone
EventSemaphore waits in a post-pass.
"""

import os
import sys

import numpy as np

for _p in (
    "/root/.axon_site",
    "/root/.axon_site/_ro/trn_rl_repo",
    "/root/.axon_site/_ro/pypackages",
    "/opt/trn_rl_repo",
    "/opt/pypackages",
):
    if os.path.isdir(_p) and _p not in sys.path:
        sys.path.append(_p)

NHID = 1024
NWORDS = 65536
NCLS = 64
CSIZE = NWORDS // NCLS  # 1024
NTOK = 8192
TPC = NTOK // NCLS      # 128
NCORES = 8
CPC = NCLS // NCORES    # 8 classes per core
TOK = CPC * TPC         # 1024 tokens per core
KT = NHID // 128        # 8 k-tiles
NF = 512                # matmul moving free dim / PSUM bank
NHALF = CSIZE // NF     # 2
NPAIR = CPC // 2        # 4 class pairs (one emb load / pw store per pair)

# consts_sb column offsets (elements)
OFF_DT = 0                      # dT: KT*TOK = 8192
OFF_CW = OFF_DT + KT * TOK      # cwT: KT*NCLS = 512
NCONST = OFF_CW + KT * NCLS     # 8704

MODE = os.environ.get("KMODE", "bf16")

# emb streamed in per-chunk loads (classes per chunk); small tail chunks
CHUNKS = (2, 2, 1, 1, 1, 1)

_CACHE = {}


def _build(mode):
    import concourse.bass as bass
    import concourse.mybir as mybir
    import concourse.tile as tile
    from bass_rust import add_dep_helper

    f32 = mybir.dt.float32
    f16 = mybir.dt.float16
    in_dt = mybir.dt.bfloat16 if mode == "bf16" else f32
    emb_bufs = 1  # each chunk has its own tag; all resident

    def mm(ap):
        return ap.bitcast(mybir.dt.float32r) if mode == "f32r" else ap

    nc = bass.Bass("TRN2", target_bir_lowering=False, debug=False)

    consts = nc.dram_tensor("consts", [128, NCONST], in_dt, kind="ExternalInput").ap()
    # flat buffer of per-chunk blocks, each [128, nc, KT, CSIZE] (p-major)
    embT = nc.dram_tensor("embT", [CPC * 128 * KT * CSIZE], in_dt, kind="ExternalInput").ap()
    pw = nc.dram_tensor("pw", [CPC, TPC, NHALF, NF], f16, kind="ExternalOutput").ap()
    pcT = nc.dram_tensor("pcT", [NCLS, NHALF, NF], f16, kind="ExternalOutput").ap()

    with tile.TileContext(nc) as tc:
        with (
            tc.tile_pool(name="const", bufs=1) as const_pool,
            tc.tile_pool(name="emb", bufs=emb_bufs) as emb_pool,
            tc.tile_pool(name="psum", bufs=4, space="PSUM") as psum_pool,
        ):
            consts_sb = const_pool.tile([128, NCONST], in_dt)
            nc.sync.dma_start(consts_sb[:], consts[:])

            out_sb = const_pool.tile([128, CPC, NHALF, NF], f16)
            oc_sb = const_pool.tile([NCLS, NHALF, NF], f16)

            def dT_lhsT(k, c):
                s = OFF_DT + k * TOK + c * TPC
                return consts_sb[:, s:s + TPC]

            def dT_rhs(k, n):
                s = OFF_DT + k * TOK + n * NF
                return consts_sb[:, s:s + NF]

            def cw_lhsT(k):
                s = OFF_CW + k * NCLS
                return consts_sb[:, s:s + NCLS]

            # ---- p_class first: only needs consts; runs while emb streams in
            # p_classT[cls, tok] = cls_w @ d.T
            for n in range(TOK // NF):
                ps = psum_pool.tile([128, NF], f32)
                for k in range(KT):
                    nc.tensor.matmul(
                        ps[:NCLS],
                        mm(cw_lhsT(k)),
                        mm(dT_rhs(k, n)),
                        start=(k == 0),
                        stop=(k == KT - 1),
                    )
                nc.scalar.copy(oc_sb[:, n, :], ps[:NCLS])

            # ---- grouped GEMM, one emb load + one pw store per chunk.
            # Small chunks at the end shrink the post-last-byte tail.
            assert sum(CHUNKS) == CPC
            c0 = 0
            for ci, ncls_chunk in enumerate(CHUNKS):
                emb_sb = emb_pool.tile([128, ncls_chunk, KT, CSIZE], in_dt,
                                       tag=f"emb{c0}")
                blk = 128 * KT * CSIZE
                src = embT[c0 * blk:(c0 + ncls_chunk) * blk].rearrange(
                    "(p c k s) -> p c k s", p=128, c=ncls_chunk, k=KT, s=CSIZE
                )
                nc.sync.dma_start(emb_sb[:], src)
                # PE-side probe: observes the emb DMA semaphore so the real
                # matmuls below need only their single PSUM-slot wait.
                probe = nc.tensor.ldweights(emb_sb[:, 0, 0, 0:TPC])
                for cc in range(ncls_chunk):
                    c = c0 + cc
                    for n in range(NHALF):
                        ps = psum_pool.tile([128, NF], f32)
                        for k in range(KT):
                            mm_inst = nc.tensor.matmul(
                                ps[:],
                                mm(dT_lhsT(k, c)),
                                mm(emb_sb[:, cc, k, n * NF:(n + 1) * NF]),
                                start=(k == 0),
                                stop=(k == KT - 1),
                            )
                            if cc == 0 and n == 0 and k == 0:
                                add_dep_helper(
                                    mm_inst.ins, probe.ins, sync=False,
                                    reason="first matmul of chunk after emb probe",
                                )
                        nc.scalar.copy(out_sb[:, c, n, :], ps[:])
                # last store rides the HWDGE ring: ~0.6us completion receipt
                # (vs ~2us SWDGE) and it is fully exposed in the kernel tail
                st_eng = nc.sync if ci == len(CHUNKS) - 1 else nc.gpsimd
                st_eng.dma_start(
                    pw[c0:c0 + ncls_chunk].rearrange("c t n s -> t c n s"),
                    out_sb[:, c0:c0 + ncls_chunk],
                )
                c0 += ncls_chunk
            nc.gpsimd.dma_start(pcT[:], oc_sb[:])

    _split_multiwait_drains(nc, mybir)
    return nc


def _split_multiwait_drains(nc, mybir):
    """This walrus build rejects instructions with >1 sync wait. Tile's final
    Drain carries one wait per semaphore; split them into standalone
    EventSemaphore waits appended to the preceding block (which executes
    immediately before the end-block drain)."""
    f = nc.m.functions[0]
    blocks = list(f.blocks)
    by_name = {b.name: b for b in blocks}
    for b in blocks:
        insts = list(b.instructions)
        if not insts:
            continue
        first = insts[0]
        if type(first).__name__ != "InstDrain":
            continue
        si = first.sync_info
        if si is None or len(si.on_wait) <= 1:
            continue
        assert b.name.endswith("_end"), b.name
        body = by_name[b.name[:-len("_end")]]
        for j, w in enumerate(si.on_wait):
            ev = mybir.InstEventSemaphore(
                name=f"{first.name}-wait{j}", engine=first.engine
            )
            ev.sync_info = mybir.SyncInfo(on_wait=[w], on_update=[])
            body.add_instruction(ev)
        first.sync_info = mybir.SyncInfo(on_wait=[], on_update=list(si.on_update))


def _get_nc(mode):
    if mode not in _CACHE:
        _CACHE[mode] = _build(mode)
    return _CACHE[mode]


def _prep_inputs(input, cls_w, word_emb, within_batch_idx, cluster, mode):
    """Host-side shard + layout. Returns (in_maps, row_ids_per_core)."""
    if mode == "bf16":
        import ml_dtypes
        np_in = ml_dtypes.bfloat16
    else:
        np_in = np.float32

    input = np.asarray(input, np.float32)
    cls_w = np.asarray(cls_w, np.float32)
    word_emb = np.asarray(word_emb, np.float32)
    idx = np.asarray(within_batch_idx).astype(np.int64)
    clu = np.asarray(cluster).astype(np.int64)

    # replicated cls decoder, [hid, cls] k-tiled: [128, KT, NCLS]
    cwT_h = np.ascontiguousarray(cls_w.T.reshape(KT, 128, NCLS).transpose(1, 0, 2))

    in_maps = []
    rows_per_core = []
    for i in range(NCORES):
        crows = idx[i * CPC:(i + 1) * CPC].reshape(-1)          # [TOK]
        rows_per_core.append(crows)
        d = input[crows]                                        # [TOK, NHID]
        dT_h = d.T.reshape(KT, 128, TOK).transpose(1, 0, 2)     # [128, KT, TOK]

        wrows = clu[i * CPC:(i + 1) * CPC].reshape(-1)          # [CPC*CSIZE]
        e = word_emb[wrows]                                     # [CPC*CSIZE, NHID]
        # flat per-chunk blocks, each [128, nc, KT, CSIZE] (p-major)
        chunks = []
        c0 = 0
        for ncls_chunk in CHUNKS:
            ec = e[c0 * CSIZE:(c0 + ncls_chunk) * CSIZE]        # [nc*CSIZE, NHID]
            chunks.append(np.ascontiguousarray(
                ec.reshape(ncls_chunk, CSIZE, KT, 128).transpose(3, 0, 2, 1)
            ).reshape(-1))
            c0 += ncls_chunk
        embT_h = np.concatenate(chunks).astype(np_in)

        consts_h = np.empty((128, NCONST), np.float32)
        consts_h[:, OFF_DT:OFF_DT + KT * TOK] = dT_h.reshape(128, KT * TOK)
        consts_h[:, OFF_CW:OFF_CW + KT * NCLS] = cwT_h.reshape(128, KT * NCLS)

        in_maps.append({
            "consts": consts_h.astype(np_in),
            "embT": embT_h,
        })
    return in_maps, rows_per_core


def _run(in_maps, mode, trace=False, trace_cores=None):
    from concourse.bass_utils import run_bass_kernel_spmd

    nc = _get_nc(mode)
    return run_bass_kernel_spmd(
        nc, in_maps, list(range(NCORES)), trace=trace, trace_cores=trace_cores
    )


def kernel(input, cls_w, cls_b, word_emb, word_bias,
           within_batch_idx, cluster, _trace=False, _mode=None, _trace_cores=None):
    mode = _mode or MODE
    in_maps, rows_per_core = _prep_inputs(
        input, cls_w, word_emb, within_batch_idx, cluster, mode)

    res = _run(in_maps, mode, trace=_trace, trace_cores=_trace_cores)

    p_words = np.empty((NCLS, TPC, CSIZE), np.float32)
    p_class = np.empty((NTOK, NCLS), np.float32)
    for i in range(NCORES):
        out = res.results[i]
        # pw [CPC, TPC, NHALF, NF] -> [CPC, TPC, CSIZE]
        p_words[i * CPC:(i + 1) * CPC] = (
            out["pw"].astype(np.float32).reshape(CPC, TPC, CSIZE)
        )
        # pcT [NCLS, NHALF, NF] -> [NCLS, TOK] -> scatter rows
        pct = out["pcT"].astype(np.float32).reshape(NCLS, TOK)
        p_class[rows_per_core[i]] = pct.T

    # biases (0.5% of FLOPs) applied on host in fp32
    word_bias = np.asarray(word_bias, np.float32)
    cls_b = np.asarray(cls_b, np.float32)
    clu = np.asarray(cluster).astype(np.int64)
    p_words += word_bias[clu][:, None, :]
    p_class += cls_b[None, :]

    # safety net: if within_batch_idx is not a permutation of all rows,
    # p_class rows outside it were never computed on-device — fill densely.
    idx_flat = np.asarray(within_batch_idx).astype(np.int64).reshape(-1)
    if np.unique(idx_flat).size != NTOK:
        p_class = (np.asarray(input, np.float32) @
                   np.asarray(cls_w, np.float32).T + cls_b[None, :])

    kernel.last_results = res
    return p_class, p_words
